# revision 1
# baseline (speedup 1.0000x reference)
import numpy as np
import concourse.bass as bass
import concourse.mybir as mybir
import concourse.tile as tile
from concourse import bacc
from concourse.bass_utils import run_bass_kernel_spmd

B, S, D, H, DH = 4, 2048, 768, 12, 64
HPC = 6          # heads per core
PAIRS = HPC // 2
THETA = 10000.0
N_CORES = 8
F32 = mybir.dt.float32
F32R = mybir.dt.float32r
VW = HPC * 65    # V tile width per t-block: 6 heads x (64 + ones col)

_NC = None


def build_nc(with_collective=True):
    nc = bacc.Bacc("TRN2", target_bir_lowering=False, debug=False,
                   num_devices=N_CORES)
    xT = nc.dram_tensor("xT", [D, S], F32R, kind="ExternalInput")
    wqT = nc.dram_tensor("wqT", [D, 384], F32R, kind="ExternalInput")
    wkT = nc.dram_tensor("wkT", [D, 384], F32R, kind="ExternalInput")
    wv = nc.dram_tensor("wv", [D, VW], F32R, kind="ExternalInput")
    wo = nc.dram_tensor("wo", [384, D], F32R, kind="ExternalInput")
    cosd = nc.dram_tensor("cos", [128, S], F32, kind="ExternalInput")
    sind = nc.dram_tensor("sin", [128, S], F32, kind="ExternalInput")
    maskd = nc.dram_tensor("mask", [128, 4 * 512], F32, kind="ExternalInput")
    onesd = nc.dram_tensor("ones", [1, 64], F32R, kind="ExternalInput")
    out = nc.dram_tensor("out", [S, D], F32, kind="ExternalOutput")

    with tile.TileContext(nc) as tc:
        with tc.tile_pool(name="persist", bufs=1) as pp, \
             tc.tile_pool(name="dram", bufs=1, space="DRAM") as dpool:
            sb_q = [pp.tile([128, S], F32R, name=f"sb_q{i}") for i in range(PAIRS)]
            sb_k = [pp.tile([128, S], F32R, name=f"sb_k{i}") for i in range(PAIRS)]
            sb_v = pp.tile([128, 16 * VW], F32R)
            sb_mask = pp.tile([128, 128], F32)
            sb_ones = pp.tile([1, 64], F32R)
            ones128 = pp.tile([128, 1], F32)
            nc.vector.memset(ones128[:], 1.0)
            bounce_in = dpool.tile([S, D], F32)
            bounce_out = dpool.tile([S, D], F32)

            nc.sync.dma_start(sb_mask[:], maskd[:, 0:128])
            nc.sync.dma_start(sb_ones[:], onesd[:])

            # ---- phase A/B: projections + RoPE, streaming x by col-block ----
            with tc.tile_pool(name="stage", bufs=1) as sp, \
                 tc.tile_pool(name="xp", bufs=2) as xp, \
                 tc.tile_pool(name="pqc", bufs=3, space="PSUM") as pqc, \
                 tc.tile_pool(name="pv", bufs=2, space="PSUM") as pv, \
                 tc.tile_pool(name="rtmp", bufs=1) as rtmp:
                sb_wq = sp.tile([128, 6 * 384], F32R)
                sb_wk = sp.tile([128, 6 * 384], F32R)
                sb_wv = sp.tile([128, 6 * VW], F32R)
                sb_cos = sp.tile([128, S], F32)
                sb_sin = sp.tile([128, S], F32)
                uh = sp.tile([128, 2 * PAIRS * 1024], F32)

                for ck in range(6):
                    for sb_w, wd in ((sb_wq, wqT), (sb_wk, wkT)):
                        nc.sync.dma_start(sb_w[:, ck * 384:(ck + 1) * 384],
                                          wd[ck * 128:(ck + 1) * 128, :])
                    nc.sync.dma_start(sb_wv[:, ck * VW:(ck + 1) * VW],
                                      wv[ck * 128:(ck + 1) * 128, :])
                nc.sync.dma_start(sb_cos[:], cosd[:])
                nc.sync.dma_start(sb_sin[:], sind[:])

                for tt in range(4):
                    xcol = xp.tile([128, 6 * 512], F32R)
                    for ck in range(6):
                        nc.sync.dma_start(
                            xcol[:, ck * 512:(ck + 1) * 512],
                            xT[ck * 128:(ck + 1) * 128,
                               tt * 512:(tt + 1) * 512])
                    csl = sb_cos[:, tt * 512:(tt + 1) * 512]
                    ssl = sb_sin[:, tt * 512:(tt + 1) * 512]
                    half = tt % 2
                    for wi, (wc, dst) in enumerate(((sb_wq, sb_q),
                                                    (sb_wk, sb_k))):
                        for p in range(PAIRS):
                            pc = pqc.tile([128, 512], F32)
                            for ck in range(6):
                                xs = xcol[:, ck * 512:(ck + 1) * 512]
                                nc.tensor.matmul(
                                    pc[:],
                                    wc[:, ck * 384 + p * 128:
                                          ck * 384 + (p + 1) * 128],
                                    xs, start=(ck == 0), stop=(ck == 5))
                            nc.vector.tensor_mul(
                                dst[p][:, tt * 512:(tt + 1) * 512],
                                pc[:], csl)
                            ub = wi * 3072 + p * 1024 + half * 512
                            nc.vector.tensor_mul(
                                uh[:, ub:ub + 512], pc[:], ssl)
                    for tj in range(4):
                        tb = tt * 4 + tj
                        pvt = pv.tile([128, VW], F32)
                        for ck in range(6):
                            nc.tensor.matmul(
                                pvt[:],
                                xcol[:, ck * 512 + tj * 128:
                                        ck * 512 + tj * 128 + 128],
                                sb_wv[:, ck * VW:(ck + 1) * VW],
                                start=(ck == 0), stop=(ck == 5))
                        with nc.allow_low_precision(reason="f32r V store"):
                            nc.vector.tensor_copy(
                                sb_v[:, tb * VW:(tb + 1) * VW], pvt[:])
                        for h in range(HPC):
                            col = tb * VW + h * 65 + 64
                            nc.scalar.copy(sb_v[:, col:col + 1], ones128[:])
                    if tt % 2 == 1:
                        base = (tt - 1) * 512
                        swf = rtmp.tile([128, 2 * PAIRS * 1024], F32)
                        for g in range(4):
                            nc.sync.dma_start(
                                swf[g * 32:(g + 1) * 32, :],
                                uh[(g ^ 1) * 32:((g ^ 1) + 1) * 32, :])
                        for wi, dsts in ((0, sb_q), (1, sb_k)):
                            for p in range(PAIRS):
                                sl = swf[:, wi * 3072 + p * 1024:
                                            wi * 3072 + (p + 1) * 1024]
                                nc.vector.tensor_add(
                                    dsts[p][:, base:base + 1024],
                                    dsts[p][:, base:base + 1024], sl)

            # ---- phases C/D: attention + output projection ----
            with tc.tile_pool(name="late", bufs=1) as lp:
                sb_ctx = [lp.tile([128, S], F32R, name=f"sb_ctx{i}") for i in range(3)]
                sb_wo = lp.tile([128, 3 * D], F32R)
                for ci in range(3):
                    nc.sync.dma_start(sb_wo[:, ci * D:(ci + 1) * D],
                                      wo[ci * 128:(ci + 1) * 128, :])

                with tc.tile_pool(name="pscore", bufs=3, space="PSUM") as pps, \
                     tc.tile_pool(name="pctx", bufs=2, space="PSUM") as ppc, \
                     tc.tile_pool(name="pbr", bufs=1, space="PSUM") as pbp, \
                     tc.tile_pool(name="po", bufs=2, space="PSUM") as po, \
                     tc.tile_pool(name="et", bufs=3) as ep, \
                     tc.tile_pool(name="ot", bufs=2) as ot, \
                     tc.tile_pool(name="nrm", bufs=2) as nrm:

                    def score_block(h, p, off, qt, kb):
                        # diag blocks (j>=0): cols below j*128 are fully
                        # masked -> skip them in matmul/exp; only the first
                        # surviving 128-col chunk needs the tril mask
                        j = kb - 4 * qt
                        lo = max(j, 0) * 128
                        psc = pps.tile([128, 512], F32)
                        nc.tensor.matmul(
                            psc[:, lo:],
                            sb_k[p][off:off + 64, kb * 128:(kb + 1) * 128],
                            sb_q[p][off:off + 64,
                                    qt * 512 + lo:(qt + 1) * 512],
                            start=True, stop=True)
                        et = ep.tile([128, 512], F32R)
                        nc.scalar.activation(et[:, lo:], psc[:, lo:],
                                             mybir.ActivationFunctionType.Exp)
                        if j >= 0:
                            nc.vector.tensor_mul(
                                et[:, lo:lo + 128], et[:, lo:lo + 128],
                                sb_mask[:, 0:128])
                        return et, lo

                    for qt in range(4):
                        for h in range(HPC):
                            p, off = h // 2, (h % 2) * 64
                            pctx = ppc.tile([65, 512], F32)
                            nkb = 4 * qt + 4
                            prev, plo = score_block(h, p, off, qt, 0)
                            for kb in range(1, nkb + 1):
                                if kb < nkb:
                                    nxt, nlo = score_block(h, p, off, qt, kb)
                                nc.tensor.matmul(
                                    pctx[:, plo:],
                                    sb_v[:, (kb - 1) * VW + h * 65:
                                            (kb - 1) * VW + h * 65 + 65],
                                    prev[:, plo:],
                                    start=(kb == 1), stop=(kb == nkb),
                                    skip_group_check=True)
                                if kb < nkb:
                                    prev, plo = nxt, nlo
                            rc = nrm.tile([1, 512], F32R)
                            with nc.allow_low_precision(
                                    reason="f32r feed to broadcast matmul"):
                                nc.vector.reciprocal(rc[:], pctx[64:65, :])
                            pbr = pbp.tile([64, 512], F32)
                            nc.tensor.matmul(pbr[:], sb_ones[:], rc[:],
                                             start=True, stop=True)
                            sc = nrm.tile([64, 512], F32)
                            nc.vector.tensor_copy(sc[:], pctx[0:64, :])
                            nc.vector.tensor_mul(
                                sb_ctx[p][off:off + 64,
                                          qt * 512:(qt + 1) * 512],
                                sc[:], pbr[:])
                        for tj in range(4):
                            tb = qt * 4 + tj
                            obuf = ot.tile([128, D], F32)
                            for nn2 in range(2):
                                pot = po.tile([128, 384], F32)
                                for ci in range(3):
                                    nc.tensor.matmul(
                                        pot[:],
                                        sb_ctx[ci][:, tb * 128:(tb + 1) * 128],
                                        sb_wo[:, ci * D + nn2 * 384:
                                                ci * D + nn2 * 384 + 384],
                                        start=(ci == 0), stop=(ci == 2))
                                nc.vector.tensor_copy(
                                    obuf[:, nn2 * 384:(nn2 + 1) * 384], pot[:])
                            nc.sync.dma_start(
                                bounce_in[tb * 128:(tb + 1) * 128, :],
                                obuf[:])

            if with_collective:
                nc.gpsimd.collective_compute(
                    "AllReduce", mybir.AluOpType.add,
                    replica_groups=[[0, 1], [2, 3], [4, 5], [6, 7]],
                    ins=[bounce_in.opt()], outs=[bounce_out.opt()])
                nc.sync.dma_start(out[:], bounce_out[:])
            else:
                nc.sync.dma_start(out[:], bounce_in[:])
    nc.compile()
    return nc


def make_in_maps(x, w_q, w_k, w_v, w_o, token_positions):
    even = np.arange(0, 64, 2)
    odd = np.arange(1, 64, 2)
    perm_eo = np.concatenate([even, odd])
    pos = np.asarray(token_positions).astype(np.float32)
    inv = THETA ** (-np.arange(32, dtype=np.float32) / 32.0)
    ang = inv[:, None] * pos[None, :]
    c32 = np.cos(ang).astype(np.float32)
    s32 = np.sin(ang).astype(np.float32)
    cosd = np.tile(c32, (4, 1))
    sind = np.concatenate([s32, -s32, s32, -s32], axis=0)
    kloc = np.arange(128)[:, None]
    qloc = np.arange(512)[None, :]
    maskd = np.concatenate(
        [(kloc + j * 128 <= qloc).astype(np.float32) for j in range(4)],
        axis=1)
    onesd = np.ones((1, 64), np.float32)
    xn = np.asarray(x, dtype=np.float32)
    wqn = np.asarray(w_q, dtype=np.float32)
    wkn = np.asarray(w_k, dtype=np.float32)
    wvn = np.asarray(w_v, dtype=np.float32)
    won = np.asarray(w_o, dtype=np.float32)
    in_maps = []
    for c in range(N_CORES):
        b, hg = c // 2, c % 2
        heads = hg * HPC + np.arange(HPC)
        rows_eo = (heads[:, None] * 64 + perm_eo[None, :]).reshape(-1)
        wv_r = np.zeros((D, VW), np.float32)
        for h in range(HPC):
            g = hg * HPC + h
            wv_r[:, h * 65:h * 65 + 64] = wvn[g * 64:(g + 1) * 64, :].T
        in_maps.append({
            "xT": np.ascontiguousarray(xn[b].T),
            "wqT": np.ascontiguousarray((wqn[rows_eo] * 0.125).T),
            "wkT": np.ascontiguousarray(wkn[rows_eo].T),
            "wv": wv_r,
            "wo": np.ascontiguousarray(won[:, hg * 384:(hg + 1) * 384].T),
            "cos": cosd,
            "sin": sind,
            "mask": maskd,
            "ones": onesd,
        })
    return in_maps


def kernel(x, w_q, w_k, w_v, w_o, token_positions):
    global _NC
    if _NC is None:
        _NC = build_nc()
    in_maps = make_in_maps(x, w_q, w_k, w_v, w_o, token_positions)
    res = run_bass_kernel_spmd(_NC, in_maps, core_ids=list(range(N_CORES)))
    return np.stack([res.results[2 * b]["out"] for b in range(B)], axis=0)



# revision 5
# speedup vs baseline: 1.0471x; 1.0471x over previous
import numpy as np
import concourse.bass as bass
import concourse.mybir as mybir
import concourse.tile as tile
from concourse import bacc
from concourse.bass_utils import run_bass_kernel_spmd

B, S, D, H, DH = 4, 2048, 768, 12, 64
HPC = 6          # heads per core
PAIRS = HPC // 2
THETA = 10000.0
N_CORES = 8
F32 = mybir.dt.float32
F32R = mybir.dt.float32r
F16 = mybir.dt.float16
VW = HPC * 65    # V tile width per t-block: 6 heads x (64 + ones col)

# stream_shuffle swaps the 16-row even/odd halves within each 32-partition
# quadrant (RoPE rotate-half, see make_in_maps for the row layout)
SHUF_MASK = [(i + 16) % 32 for i in range(32)]

_NC = None


def build_nc(with_collective=True):
    nc = bacc.Bacc("TRN2", target_bir_lowering=False, debug=False,
                   num_devices=N_CORES)
    xT = nc.dram_tensor("xT", [D, S], F32R, kind="ExternalInput")
    wqT = nc.dram_tensor("wqT", [D, 384], F32R, kind="ExternalInput")
    wkT = nc.dram_tensor("wkT", [D, 384], F32R, kind="ExternalInput")
    wv = nc.dram_tensor("wv", [D, VW], F32R, kind="ExternalInput")
    wo = nc.dram_tensor("wo", [384, D], F16, kind="ExternalInput")
    cosd = nc.dram_tensor("cos", [128, S], F32, kind="ExternalInput")
    sind = nc.dram_tensor("sin", [128, S], F32, kind="ExternalInput")
    maskd = nc.dram_tensor("mask", [128, 128], F16, kind="ExternalInput")
    idend = nc.dram_tensor("iden", [128, 128], F16, kind="ExternalInput")
    out = nc.dram_tensor("out", [S, D], F32, kind="ExternalOutput")

    with tile.TileContext(nc) as tc:
        with tc.tile_pool(name="persist", bufs=1) as pp, \
             tc.tile_pool(name="dram", bufs=1, space="DRAM") as dpool:
            sb_q = [pp.tile([128, S], F16, name=f"sb_q{i}") for i in range(PAIRS)]
            sb_k = [pp.tile([128, S], F16, name=f"sb_k{i}") for i in range(PAIRS)]
            sb_v = pp.tile([128, 16 * VW], F16)
            sb_ctxT = [pp.tile([128, S], F16, name=f"sb_ctxT{i}")
                       for i in range(PAIRS)]
            sb_wo = pp.tile([128, 3 * D], F16)
            sb_mask = pp.tile([128, 128], F16)
            sb_iden = pp.tile([128, 128], F16)
            bounce_in = dpool.tile([S, D], F32)
            bounce_out = dpool.tile([S, D], F32)

            nc.sync.dma_start(sb_mask[:], maskd[:])
            nc.sync.dma_start(sb_iden[:], idend[:])
            for ci in range(3):
                nc.sync.dma_start(sb_wo[:, ci * D:(ci + 1) * D],
                                  wo[ci * 128:(ci + 1) * 128, :])
            # ones columns of V (softmax denominator trick): one strided memset
            nc.gpsimd.memset(sb_v[:, 64::65], 1.0)

            # ---- phase A: QKV projections + RoPE, streaming x by col-block --
            with tc.tile_pool(name="stage", bufs=1) as sp, \
                 tc.tile_pool(name="xp", bufs=2) as xp, \
                 tc.tile_pool(name="pqc", bufs=3, space="PSUM") as pqc, \
                 tc.tile_pool(name="pv", bufs=2, space="PSUM") as pv, \
                 tc.tile_pool(name="ru", bufs=3) as ru:
                sb_wq = sp.tile([128, 6 * 384], F32R)
                sb_wk = sp.tile([128, 6 * 384], F32R)
                sb_wv = sp.tile([128, 6 * VW], F32R)
                sb_cos = sp.tile([128, S], F32)
                sb_sin = sp.tile([128, S], F32)

                for ck in range(6):
                    for sb_w, wd in ((sb_wq, wqT), (sb_wk, wkT)):
                        nc.sync.dma_start(sb_w[:, ck * 384:(ck + 1) * 384],
                                          wd[ck * 128:(ck + 1) * 128, :])
                    nc.sync.dma_start(sb_wv[:, ck * VW:(ck + 1) * VW],
                                      wv[ck * 128:(ck + 1) * 128, :])
                nc.sync.dma_start(sb_cos[:], cosd[:])
                nc.sync.dma_start(sb_sin[:], sind[:])

                for tt in range(4):
                    xcol = xp.tile([128, 6 * 512], F32R)
                    for ck in range(6):
                        nc.sync.dma_start(
                            xcol[:, ck * 512:(ck + 1) * 512],
                            xT[ck * 128:(ck + 1) * 128,
                               tt * 512:(tt + 1) * 512])
                    csl = sb_cos[:, tt * 512:(tt + 1) * 512]
                    ssl = sb_sin[:, tt * 512:(tt + 1) * 512]
                    for wc, dst in ((sb_wq, sb_q), (sb_wk, sb_k)):
                        for p in range(PAIRS):
                            pc = pqc.tile([128, 512], F32)
                            for ck in range(6):
                                nc.tensor.matmul(
                                    pc[:],
                                    wc[:, ck * 384 + p * 128:
                                          ck * 384 + (p + 1) * 128],
                                    xcol[:, ck * 512:(ck + 1) * 512],
                                    start=(ck == 0), stop=(ck == 5))
                            dsl = dst[p][:, tt * 512:(tt + 1) * 512]
                            uh = ru.tile([128, 512], F16)
                            uhs = ru.tile([128, 512], F16)
                            with nc.allow_low_precision(reason="f16 q/k"):
                                nc.vector.tensor_mul(dsl, pc[:], csl)
                                nc.gpsimd.tensor_mul(uh[:], pc[:], ssl)
                                nc.vector.stream_shuffle(uhs[:], uh[:],
                                                         SHUF_MASK)
                                nc.vector.tensor_add(dsl, dsl, uhs[:])
                    for tj in range(4):
                        tb = tt * 4 + tj
                        pvt = pv.tile([128, VW], F32)
                        for ck in range(6):
                            nc.tensor.matmul(
                                pvt[:],
                                xcol[:, ck * 512 + tj * 128:
                                        ck * 512 + tj * 128 + 128],
                                sb_wv[:, ck * VW:(ck + 1) * VW],
                                start=(ck == 0), stop=(ck == 5))
                        with nc.allow_low_precision(reason="f16 V store"):
                            # 64-wide per-head slices only; ones cols preset
                            for h in range(HPC):
                                nc.gpsimd.tensor_copy(
                                    sb_v[:, tb * VW + h * 65:
                                            tb * VW + h * 65 + 64],
                                    pvt[:, h * 65:h * 65 + 64])

            # ---- phase C: attention (flipped ctx) + output projection ------
            with tc.tile_pool(name="pscore", bufs=2, space="PSUM") as pps, \
                 tc.tile_pool(name="et", bufs=2) as etp, \
                 tc.tile_pool(name="pctx", bufs=2, space="PSUM") as pcx, \
                 tc.tile_pool(name="ptr", bufs=2, space="PSUM") as ptp, \
                 tc.tile_pool(name="po", bufs=2, space="PSUM") as pop, \
                 tc.tile_pool(name="nrm", bufs=6) as nrm, \
                 tc.tile_pool(name="cxq", bufs=3) as cxq, \
                 tc.tile_pool(name="ot", bufs=2) as otp:

                def scores(h, qt, et):
                    # score+exp blocks for head h, q columns [qt*512,(qt+1)*512)
                    p, off = h // 2, (h % 2) * 64
                    nkb = 4 * qt + 4
                    for kb in range(nkb):
                        j = kb - 4 * qt
                        lo = max(j, 0) * 128
                        psc = pps.tile([128, 512], F32)
                        nc.tensor.matmul(
                            psc[:, lo:],
                            sb_k[p][off:off + 64, kb * 128:(kb + 1) * 128],
                            sb_q[p][off:off + 64,
                                    qt * 512 + lo:(qt + 1) * 512],
                            start=True, stop=True)
                        esl = et[:, kb * 512 + lo:(kb + 1) * 512]
                        with nc.allow_low_precision(reason="f16 attn"):
                            nc.scalar.activation(
                                esl, psc[:, lo:],
                                mybir.ActivationFunctionType.Exp)
                            if j >= 0:
                                msl = et[:, kb * 512 + lo:kb * 512 + lo + 128]
                                nc.vector.tensor_mul(msl, msl, sb_mask[:])

                def ctx(h, qt, et, ctxq):
                    # flipped context: [128q, 65] accumulation per 128-q
                    # chunk; 4 chunks share one PSUM bank (65-col slots)
                    pct = pcx.tile([128, 260], F32)
                    for qc in range(4):
                        qg = 4 * qt + qc
                        psl = pct[:, qc * 65:(qc + 1) * 65]
                        for kb in range(qg + 1):
                            nc.tensor.matmul(
                                psl,
                                et[:, kb * 512 + qc * 128:
                                      kb * 512 + qc * 128 + 128],
                                sb_v[:, kb * VW + h * 65:kb * VW + h * 65 + 65],
                                start=(kb == 0), stop=(kb == qg),
                                skip_group_check=True)
                        rc = nrm.tile([128, 1], F32)
                        nc.vector.reciprocal(rc[:], pct[:, qc * 65 + 64:
                                                           qc * 65 + 65])
                        with nc.allow_low_precision(reason="f16 ctx"):
                            nc.vector.tensor_scalar_mul(
                                out=ctxq[qc][:, h * 64:(h + 1) * 64],
                                in0=pct[:, qc * 65:qc * 65 + 64],
                                scalar1=rc[:])

                for qt in range(4):
                    etiles = []
                    ctxq = [cxq.tile([128, 384], F16, name=f"cxq{qt}_{qc}")
                            for qc in range(4)]
                    for h in range(HPC):
                        et = etp.tile([128, 16 * 512], F16)
                        scores(h, qt, et)
                        etiles.append(et)
                        if h > 0:     # ctx for previous head (hides exp)
                            ctx(h - 1, qt, etiles[h - 1], ctxq)
                    ctx(HPC - 1, qt, etiles[HPC - 1], ctxq)

                    # transpose ctx [q, d] -> [d, q] for the O projection
                    for qc in range(4):
                        ptr = ptp.tile([128, 384], F16)
                        for p3 in range(PAIRS):
                            nc.tensor.transpose(
                                ptr[:, p3 * 128:(p3 + 1) * 128],
                                ctxq[qc][:, p3 * 128:(p3 + 1) * 128],
                                sb_iden[:])
                            with nc.allow_low_precision(reason="f16 ctxT"):
                                nc.gpsimd.tensor_copy(
                                    sb_ctxT[p3][:, (4 * qt + qc) * 128:
                                                 (4 * qt + qc + 1) * 128],
                                    ptr[:, p3 * 128:(p3 + 1) * 128])
                    # output projection for this qt
                    for tj in range(4):
                        tb = qt * 4 + tj
                        obuf = otp.tile([128, D], F32)
                        for half in range(2):
                            po = pop.tile([128, 384], F32)
                            for ci in range(3):
                                nc.tensor.matmul(
                                    po[:],
                                    sb_ctxT[ci][:, tb * 128:(tb + 1) * 128],
                                    sb_wo[:, ci * D + half * 384:
                                             ci * D + half * 384 + 384],
                                    start=(ci == 0), stop=(ci == 2))
                            nc.vector.tensor_copy(
                                obuf[:, half * 384:(half + 1) * 384], po[:])
                        nc.sync.dma_start(
                            bounce_in[tb * 128:(tb + 1) * 128, :], obuf[:])

            if with_collective:
                nc.gpsimd.collective_compute(
                    "AllReduce", mybir.AluOpType.add,
                    replica_groups=[[0, 1], [2, 3], [4, 5], [6, 7]],
                    ins=[bounce_in.opt()], outs=[bounce_out.opt()])
                nc.sync.dma_start(out[:], bounce_out[:])
            else:
                nc.sync.dma_start(out[:], bounce_in[:])
    nc.compile()
    return nc


def make_in_maps(x, w_q, w_k, w_v, w_o, token_positions):
    # RoPE row layout: per 64-dim head-half, rows are 2 quadrant-pairs of
    # [16 even dims | 16 odd dims]; stream_shuffle swaps the 16-row halves
    # within each 32-row quadrant.
    r64 = np.arange(64)
    perm64 = 2 * (16 * (r64 // 32) + (r64 % 16)) + ((r64 % 32) >= 16)
    pos = np.asarray(token_positions).astype(np.float32)
    inv = THETA ** (-np.arange(32, dtype=np.float32) / 32.0)
    ang = inv[:, None] * pos[None, :]                       # [32 freqs, S]
    c32 = np.cos(ang).astype(np.float32)
    s32 = np.sin(ang).astype(np.float32)
    r128 = np.arange(128)
    fi = 16 * ((r128 // 32) % 2) + (r128 % 16)              # freq per row
    sgn = np.where((r128 % 32) < 16, 1.0, -1.0).astype(np.float32)
    cosd = c32[fi]                                          # [128, S]
    sind = s32[fi] * sgn[:, None]
    kloc = np.arange(128)[:, None]
    qloc = np.arange(128)[None, :]
    maskd = (kloc <= qloc).astype(np.float16)
    idend = np.eye(128, dtype=np.float16)
    xn = np.asarray(x, dtype=np.float32)
    wqn = np.asarray(w_q, dtype=np.float32)
    wkn = np.asarray(w_k, dtype=np.float32)
    wvn = np.asarray(w_v, dtype=np.float32)
    won = np.asarray(w_o, dtype=np.float32)
    in_maps = []
    for c in range(N_CORES):
        b, hg = c // 2, c % 2
        heads = hg * HPC + np.arange(HPC)
        rows_eo = (heads[:, None] * 64 + perm64[None, :]).reshape(-1)
        wv_r = np.zeros((D, VW), np.float32)
        for h in range(HPC):
            g = hg * HPC + h
            wv_r[:, h * 65:h * 65 + 64] = wvn[g * 64:(g + 1) * 64, :].T
        wo_cols = (heads[:, None] * 64 + np.arange(64)[None, :]).reshape(-1)
        wo_r = np.ascontiguousarray(won[:, wo_cols].T).astype(np.float16)
        in_maps.append({
            "xT": np.ascontiguousarray(xn[b].T),
            "wqT": np.ascontiguousarray((wqn[rows_eo] * 0.125).T),
            "wkT": np.ascontiguousarray(wkn[rows_eo].T),
            "wv": wv_r,
            "wo": wo_r,
            "cos": cosd,
            "sin": sind,
            "mask": maskd,
            "iden": idend,
        })
    return in_maps


def kernel(x, w_q, w_k, w_v, w_o, token_positions):
    global _NC
    if _NC is None:
        _NC = build_nc()
    in_maps = make_in_maps(x, w_q, w_k, w_v, w_o, token_positions)
    res = run_bass_kernel_spmd(_NC, in_maps, core_ids=list(range(N_CORES)))
    return np.stack([res.results[2 * b]["out"] for b in range(B)], axis=0)


# revision 13
# speedup vs baseline: 1.0938x; 1.0447x over previous
import numpy as np
import concourse.bass as bass
import concourse.mybir as mybir
import concourse.tile as tile
from concourse import bacc
from concourse.bass_utils import run_bass_kernel_spmd

B, S, D, H, DH = 4, 2048, 768, 12, 64
HPC = 6          # heads per core
PAIRS = HPC // 2
THETA = 10000.0
N_CORES = 8
F32 = mybir.dt.float32
F32R = mybir.dt.float32r
F16 = mybir.dt.float16
VW = HPC * 65    # V block width: 6 heads x (64 + ones col)

# stream_shuffle swaps the 16-row even/odd halves within each 32-partition
# quadrant (RoPE rotate-half, see make_in_maps for the row layout)
SHUF_MASK = [(i + 16) % 32 for i in range(32)]

_NC = None


def build_nc(with_collective=True):
    nc = bacc.Bacc("TRN2", target_bir_lowering=False, debug=False,
                   num_devices=N_CORES)
    xT = nc.dram_tensor("xT", [D, S], F32R, kind="ExternalInput")
    wqT = nc.dram_tensor("wqT", [D, 384], F32R, kind="ExternalInput")
    wkT = nc.dram_tensor("wkT", [D, 384], F32R, kind="ExternalInput")
    wv = nc.dram_tensor("wv", [D, VW], F32R, kind="ExternalInput")
    wo = nc.dram_tensor("wo", [384, D], F16, kind="ExternalInput")
    cosd = nc.dram_tensor("cos", [128, S], F16, kind="ExternalInput")
    sind = nc.dram_tensor("sin", [128, S], F16, kind="ExternalInput")
    maskd = nc.dram_tensor("mask", [128, 128], F16, kind="ExternalInput")
    idend = nc.dram_tensor("iden", [128, 128], F16, kind="ExternalInput")
    out = nc.dram_tensor("out", [S, D], F32, kind="ExternalOutput")

    with tile.TileContext(nc) as tc:
        with tc.tile_pool(name="persist", bufs=1) as pp, \
             tc.tile_pool(name="dram", bufs=1, space="DRAM") as dpool, \
             tc.tile_pool(name="xp", bufs=2) as xp, \
             tc.tile_pool(name="ru", bufs=4) as ru, \
             tc.tile_pool(name="et", bufs=3) as etp, \
             tc.tile_pool(name="nrm", bufs=6) as nrm, \
             tc.tile_pool(name="cxq", bufs=2) as cxq, \
             tc.tile_pool(name="ot", bufs=2) as otp, \
             tc.tile_pool(name="pqc", bufs=2, space="PSUM") as pqc, \
             tc.tile_pool(name="pps", bufs=2, space="PSUM") as pps, \
             tc.tile_pool(name="pcx", bufs=1, space="PSUM") as pcx, \
             tc.tile_pool(name="ptr", bufs=1, space="PSUM") as ptp:
            sb_q = [pp.tile([128, S], F16, name=f"sb_q{i}") for i in range(PAIRS)]
            sb_k = [pp.tile([128, S], F16, name=f"sb_k{i}") for i in range(PAIRS)]
            sb_v = pp.tile([128, 96, 65], F16)
            sb_ctxT = [pp.tile([128, S], F16, name=f"sb_ctxT{i}")
                       for i in range(PAIRS)]
            sb_wo = pp.tile([128, 3 * D], F16)
            sb_mask = pp.tile([128, 128], F16)
            sb_iden = pp.tile([128, 128], F16)
            sb_wq = pp.tile([128, 6 * 384], F32R)
            sb_wk = pp.tile([128, 6 * 384], F32R)
            sb_wv = pp.tile([128, 6 * VW], F32R)
            sb_cos = pp.tile([128, S], F16)
            sb_sin = pp.tile([128, S], F16)
            bounce_in = dpool.tile([S, D], F32)
            bounce_out = dpool.tile([S, D], F32)

            # x stream for tt=0 issues first (gpsimd DGE trigger is cheap
            # and HWDGE serves roughly in issue order)
            xcols = [None] * 4

            def load_x(tt):
                xcols[tt] = xp.tile([128, 6 * 512], F32R, name="xcol")
                for ck in range(6):
                    nc.gpsimd.dma_start(
                        xcols[tt][:, ck * 512:(ck + 1) * 512],
                        xT[ck * 128:(ck + 1) * 128, tt * 512:(tt + 1) * 512])

            load_x(0)
            for ck in range(6):
                nc.sync.dma_start(sb_wq[:, ck * 384:(ck + 1) * 384],
                                  wqT[ck * 128:(ck + 1) * 128, :])
            nc.sync.dma_start(sb_cos[:], cosd[:])
            nc.sync.dma_start(sb_sin[:], sind[:])
            for ck in range(6):
                nc.sync.dma_start(sb_wk[:, ck * 384:(ck + 1) * 384],
                                  wkT[ck * 128:(ck + 1) * 128, :])
            for ck in range(6):
                nc.sync.dma_start(sb_wv[:, ck * VW:(ck + 1) * VW],
                                  wv[ck * 128:(ck + 1) * 128, :])
            nc.sync.dma_start(sb_mask[:], maskd[:])
            nc.sync.dma_start(sb_iden[:], idend[:])
            for ci in range(3):
                nc.sync.dma_start(sb_wo[:, ci * D:(ci + 1) * D],
                                  wo[ci * 128:(ci + 1) * 128, :])

            def proj(tt):
                xcol = xcols[tt]
                csl = sb_cos[:, tt * 512:(tt + 1) * 512]
                ssl = sb_sin[:, tt * 512:(tt + 1) * 512]
                for wc, dst in ((sb_wq, sb_q), (sb_wk, sb_k)):
                    for p in range(PAIRS):
                        pc = pqc.tile([128, 512], F32, name="pqk")
                        for ck in range(6):
                            nc.tensor.matmul(
                                pc[:],
                                wc[:, ck * 384 + p * 128:
                                      ck * 384 + (p + 1) * 128],
                                xcol[:, ck * 512:(ck + 1) * 512],
                                start=(ck == 0), stop=(ck == 5))
                        dsl = dst[p][:, tt * 512:(tt + 1) * 512]
                        uh = ru.tile([128, 512], F16)
                        uhs = ru.tile([128, 512], F16)
                        with nc.allow_low_precision(reason="f16 q/k"):
                            nc.vector.tensor_mul(dsl, pc[:], csl)
                            nc.gpsimd.tensor_mul(uh[:], pc[:], ssl)
                            nc.vector.stream_shuffle(uhs[:], uh[:], SHUF_MASK)
                            nc.vector.tensor_add(dsl, dsl, uhs[:])
                for tj in range(4):
                    tb = tt * 4 + tj
                    pvt = pqc.tile([128, 512], F32, name="pqk")
                    for ck in range(6):
                        nc.tensor.matmul(
                            pvt[:, 0:VW],
                            xcol[:, ck * 512 + tj * 128:
                                    ck * 512 + tj * 128 + 128],
                            sb_wv[:, ck * VW:(ck + 1) * VW],
                            start=(ck == 0), stop=(ck == 5))
                    with nc.allow_low_precision(reason="f16 V store"):
                        nc.gpsimd.tensor_copy(
                            sb_v[:, tb * 6:(tb + 1) * 6, :],
                            pvt[:, 0:VW].rearrange("p (a b) -> p a b", b=65))
                    nc.gpsimd.memset(sb_v[:, tb * 6:(tb + 1) * 6, 64:65], 1.0)

            def scores(h, qt, et):
                # score+exp blocks for head h, q cols [qt*512,(qt+1)*512);
                # full (below-diagonal) k-blocks run pairwise through one
                # 1024-col PSUM tile so exp covers two blocks per instruction
                p, off = h // 2, (h % 2) * 64
                qsl = sb_q[p][off:off + 64, qt * 512:(qt + 1) * 512]
                for kb2 in range(2 * qt):
                    psc = pps.tile([128, 1024], F32, name="psc")
                    for u in range(2):
                        kb = 2 * kb2 + u
                        nc.tensor.matmul(
                            psc[:, u * 512:(u + 1) * 512],
                            sb_k[p][off:off + 64, kb * 128:(kb + 1) * 128],
                            qsl, start=True, stop=True)
                    with nc.allow_low_precision(reason="f16 attn"):
                        nc.scalar.activation(
                            et[:, kb2 * 1024:(kb2 + 1) * 1024], psc[:],
                            mybir.ActivationFunctionType.Exp)
                for j in range(4):
                    kb = 4 * qt + j
                    lo = j * 128
                    psc = pps.tile([128, 1024], F32, name="psc")
                    nc.tensor.matmul(
                        psc[:, lo:512],
                        sb_k[p][off:off + 64, kb * 128:(kb + 1) * 128],
                        sb_q[p][off:off + 64,
                                qt * 512 + lo:(qt + 1) * 512],
                        start=True, stop=True)
                    esl = et[:, kb * 512 + lo:(kb + 1) * 512]
                    with nc.allow_low_precision(reason="f16 attn"):
                        nc.scalar.activation(
                            esl, psc[:, lo:512],
                            mybir.ActivationFunctionType.Exp)
                        msl = et[:, kb * 512 + lo:kb * 512 + lo + 128]
                        nc.vector.tensor_mul(msl, msl, sb_mask[:])

            def ctx(h, qt, et, ctxq):
                # flipped context: [128q, 65] accumulation per 128-q chunk;
                # 4 chunks share one PSUM bank (65-col slots)
                pct = pcx.tile([128, 260], F32)
                for qc in range(4):
                    qg = 4 * qt + qc
                    psl = pct[:, qc * 65:(qc + 1) * 65]
                    for kb in range(qg + 1):
                        nc.tensor.matmul(
                            psl,
                            et[:, kb * 512 + qc * 128:
                                  kb * 512 + qc * 128 + 128],
                            sb_v[:, kb * 6 + h, :],
                            start=(kb == 0), stop=(kb == qg),
                            skip_group_check=True)
                    rc = nrm.tile([128, 1], F32)
                    nc.vector.reciprocal(rc[:], pct[:, qc * 65 + 64:
                                                       qc * 65 + 65])
                    with nc.allow_low_precision(reason="f16 ctx"):
                        nc.vector.tensor_scalar_mul(
                            out=ctxq[qc][:, h * 64:(h + 1) * 64],
                            in0=pct[:, qc * 65:qc * 65 + 64],
                            scalar1=rc[:])

            def att(qt):
                ctxq = [cxq.tile([128, 384], F16, name=f"cxq{i}")
                        for i in range(4)]
                etiles = []
                for h in range(HPC):
                    et = etp.tile([128, 16 * 512], F16)
                    scores(h, qt, et)
                    etiles.append(et)
                    if h > 0:     # ctx for previous head (hides exp latency)
                        ctx(h - 1, qt, etiles[h - 1], ctxq)
                ctx(HPC - 1, qt, etiles[HPC - 1], ctxq)

                for qc in range(4):
                    tb = qt * 4 + qc
                    # transpose ctx [q, d] -> [d, q], then O projection
                    ptr = ptp.tile([128, 384], F16)
                    for p3 in range(PAIRS):
                        nc.tensor.transpose(
                            ptr[:, p3 * 128:(p3 + 1) * 128],
                            ctxq[qc][:, p3 * 128:(p3 + 1) * 128],
                            sb_iden[:])
                        with nc.allow_low_precision(reason="f16 ctxT"):
                            nc.gpsimd.tensor_copy(
                                sb_ctxT[p3][:, tb * 128:(tb + 1) * 128],
                                ptr[:, p3 * 128:(p3 + 1) * 128])
                    obuf = otp.tile([128, D], F32)
                    for half in range(2):
                        po = pps.tile([128, 1024], F32, name="psc")
                        for ci in range(3):
                            nc.tensor.matmul(
                                po[:, 0:384],
                                sb_ctxT[ci][:, tb * 128:(tb + 1) * 128],
                                sb_wo[:, ci * D + half * 384:
                                         ci * D + half * 384 + 384],
                                start=(ci == 0), stop=(ci == 2))
                        nc.vector.tensor_copy(
                            obuf[:, half * 384:(half + 1) * 384],
                            po[:, 0:384])
                    nc.gpsimd.dma_start(
                        bounce_in[tb * 128:(tb + 1) * 128, :], obuf[:])

            for tt in range(4):
                if tt < 3:
                    load_x(tt + 1)
                proj(tt)
                if tt >= 1:
                    att(tt - 1)
            att(3)

            if with_collective:
                nc.gpsimd.collective_compute(
                    "AllReduce", mybir.AluOpType.add,
                    replica_groups=[[0, 1], [2, 3], [4, 5], [6, 7]],
                    ins=[bounce_in.opt()], outs=[bounce_out.opt()])
                nc.sync.dma_start(out[:], bounce_out[:])
            else:
                nc.sync.dma_start(out[:], bounce_in[:])
    nc.compile()
    return nc


def make_in_maps(x, w_q, w_k, w_v, w_o, token_positions):
    # RoPE row layout: per 64-dim head-half, rows are 2 quadrant-pairs of
    # [16 even dims | 16 odd dims]; stream_shuffle swaps the 16-row halves
    # within each 32-row quadrant.
    r64 = np.arange(64)
    perm64 = 2 * (16 * (r64 // 32) + (r64 % 16)) + ((r64 % 32) >= 16)
    pos = np.asarray(token_positions).astype(np.float32)
    inv = THETA ** (-np.arange(32, dtype=np.float32) / 32.0)
    ang = inv[:, None] * pos[None, :]                       # [32 freqs, S]
    c32 = np.cos(ang).astype(np.float32)
    s32 = np.sin(ang).astype(np.float32)
    r128 = np.arange(128)
    fi = 16 * ((r128 // 32) % 2) + (r128 % 16)              # freq per row
    sgn = np.where((r128 % 32) < 16, 1.0, -1.0).astype(np.float32)
    cosd = c32[fi].astype(np.float16)                       # [128, S]
    sind = (s32[fi] * sgn[:, None]).astype(np.float16)
    kloc = np.arange(128)[:, None]
    qloc = np.arange(128)[None, :]
    maskd = (kloc <= qloc).astype(np.float16)
    idend = np.eye(128, dtype=np.float16)
    xn = np.asarray(x, dtype=np.float32)
    wqn = np.asarray(w_q, dtype=np.float32)
    wkn = np.asarray(w_k, dtype=np.float32)
    wvn = np.asarray(w_v, dtype=np.float32)
    won = np.asarray(w_o, dtype=np.float32)
    in_maps = []
    for c in range(N_CORES):
        b, hg = c // 2, c % 2
        heads = hg * HPC + np.arange(HPC)
        rows_eo = (heads[:, None] * 64 + perm64[None, :]).reshape(-1)
        wv_r = np.zeros((D, VW), np.float32)
        for h in range(HPC):
            g = hg * HPC + h
            wv_r[:, h * 65:h * 65 + 64] = wvn[g * 64:(g + 1) * 64, :].T
        wo_cols = (heads[:, None] * 64 + np.arange(64)[None, :]).reshape(-1)
        wo_r = np.ascontiguousarray(won[:, wo_cols].T).astype(np.float16)
        in_maps.append({
            "xT": np.ascontiguousarray(xn[b].T),
            "wqT": np.ascontiguousarray((wqn[rows_eo] * 0.125).T),
            "wkT": np.ascontiguousarray(wkn[rows_eo].T),
            "wv": wv_r,
            "wo": wo_r,
            "cos": cosd,
            "sin": sind,
            "mask": maskd,
            "iden": idend,
        })
    return in_maps


def kernel(x, w_q, w_k, w_v, w_o, token_positions):
    global _NC
    if _NC is None:
        _NC = build_nc()
    in_maps = make_in_maps(x, w_q, w_k, w_v, w_o, token_positions)
    res = run_bass_kernel_spmd(_NC, in_maps, core_ids=list(range(N_CORES)))
    return np.stack([res.results[2 * b]["out"] for b in range(B)], axis=0)


# revision 16
# speedup vs baseline: 1.2814x; 1.1714x over previous
import numpy as np
import concourse.bass as bass
import concourse.mybir as mybir
import concourse.tile as tile
from concourse import bacc
from concourse.bass_utils import run_bass_kernel_spmd

B, S, D, H, DH = 4, 2048, 768, 12, 64
HPC = 6          # heads per core
PAIRS = HPC // 2
THETA = 10000.0
N_CORES = 8
F32 = mybir.dt.float32
F32R = mybir.dt.float32r
F16 = mybir.dt.float16
VW = HPC * 65    # V block width: 6 heads x (64 + ones col)

# stream_shuffle swaps the 16-row even/odd halves within each 32-partition
# quadrant (RoPE rotate-half, see make_in_maps for the row layout)
SHUF_MASK = [(i + 16) % 32 for i in range(32)]

_NC = None


def interleave(main_units, extra_units):
    """Merge two unit lists, spreading extra_units evenly among main_units."""
    n, m = len(main_units), len(extra_units)
    if n == 0:
        return list(extra_units)
    res, j = [], 0
    for i, u in enumerate(main_units):
        res.append(u)
        while j < m and (j + 1) * n <= (i + 1) * m:
            res.append(extra_units[j])
            j += 1
    res.extend(extra_units[j:])
    return res


def build_nc(with_collective=True):
    nc = bacc.Bacc("TRN2", target_bir_lowering=False, debug=False,
                   num_devices=N_CORES)
    xT = nc.dram_tensor("xT", [D, S], F32R, kind="ExternalInput")
    wqT = nc.dram_tensor("wqT", [D, 384], F32R, kind="ExternalInput")
    wkT = nc.dram_tensor("wkT", [D, 384], F32R, kind="ExternalInput")
    wv = nc.dram_tensor("wv", [D, VW], F32R, kind="ExternalInput")
    wo = nc.dram_tensor("wo", [384, D], F16, kind="ExternalInput")
    cosd = nc.dram_tensor("cos", [128, S], F16, kind="ExternalInput")
    sind = nc.dram_tensor("sin", [128, S], F16, kind="ExternalInput")
    maskd = nc.dram_tensor("mask", [128, 128], F16, kind="ExternalInput")
    idend = nc.dram_tensor("iden", [128, 128], F16, kind="ExternalInput")
    out = nc.dram_tensor("out", [S, D], F32, kind="ExternalOutput")

    with tile.TileContext(nc) as tc:
        with tc.tile_pool(name="persist", bufs=1) as pp, \
             tc.tile_pool(name="dram", bufs=1, space="DRAM") as dpool, \
             tc.tile_pool(name="xp", bufs=2) as xp, \
             tc.tile_pool(name="ru", bufs=4) as ru, \
             tc.tile_pool(name="et", bufs=3) as etp, \
             tc.tile_pool(name="nrm", bufs=6) as nrm, \
             tc.tile_pool(name="cxq", bufs=2) as cxq, \
             tc.tile_pool(name="ot", bufs=2) as otp, \
             tc.tile_pool(name="pqc", bufs=2, space="PSUM") as pqc, \
             tc.tile_pool(name="pps", bufs=2, space="PSUM") as pps, \
             tc.tile_pool(name="pcx", bufs=1, space="PSUM") as pcx, \
             tc.tile_pool(name="ptr", bufs=1, space="PSUM") as ptp:
            sb_q = [pp.tile([128, S], F16, name=f"sb_q{i}") for i in range(PAIRS)]
            sb_k = [pp.tile([128, S], F16, name=f"sb_k{i}") for i in range(PAIRS)]
            sb_v = pp.tile([128, 96, 65], F16)
            sb_ctxT = [pp.tile([128, S], F16, name=f"sb_ctxT{i}")
                       for i in range(PAIRS)]
            sb_wo = pp.tile([128, 3 * D], F16)
            sb_mask = pp.tile([128, 128], F16)
            sb_iden = pp.tile([128, 128], F16)
            sb_wq = pp.tile([128, 6 * 384], F32R)
            sb_wk = pp.tile([128, 6 * 384], F32R)
            sb_wv = pp.tile([128, 6 * VW], F32R)
            sb_cos = pp.tile([128, S], F16)
            sb_sin = pp.tile([128, S], F16)
            bounce_in = dpool.tile([S, D], F32)
            bounce_out = dpool.tile([S, D], F32)
            out_dram = bounce_in if with_collective else out

            xcols = [None] * 4

            def load_x(tt):
                xcols[tt] = xp.tile([128, 6 * 512], F32R, name="xcol")
                for ck in range(6):
                    nc.sync.dma_start(
                        xcols[tt][:, ck * 512:(ck + 1) * 512],
                        xT[ck * 128:(ck + 1) * 128, tt * 512:(tt + 1) * 512])

            # startup DMA order: first QK matmul needs x0[ck]+wq[ck] pairs
            xcols[0] = xp.tile([128, 6 * 512], F32R, name="xcol")
            for ck in range(6):
                nc.sync.dma_start(
                    xcols[0][:, ck * 512:(ck + 1) * 512],
                    xT[ck * 128:(ck + 1) * 128, 0:512])
                nc.sync.dma_start(sb_wq[:, ck * 384:(ck + 1) * 384],
                                  wqT[ck * 128:(ck + 1) * 128, :])
            nc.sync.dma_start(sb_cos[:], cosd[:])
            nc.sync.dma_start(sb_sin[:], sind[:])
            for ck in range(6):
                nc.sync.dma_start(sb_wk[:, ck * 384:(ck + 1) * 384],
                                  wkT[ck * 128:(ck + 1) * 128, :])
            for ck in range(6):
                nc.sync.dma_start(sb_wv[:, ck * VW:(ck + 1) * VW],
                                  wv[ck * 128:(ck + 1) * 128, :])
            nc.sync.dma_start(sb_mask[:], maskd[:])
            nc.sync.dma_start(sb_iden[:], idend[:])
            for ci in range(3):
                nc.sync.dma_start(sb_wo[:, ci * D:(ci + 1) * D],
                                  wo[ci * 128:(ci + 1) * 128, :])

            def proj_units(tt):
                """QKV projection + RoPE for token block tt as emit-units."""
                xcol = xcols[tt]
                csl = sb_cos[:, tt * 512:(tt + 1) * 512]
                ssl = sb_sin[:, tt * 512:(tt + 1) * 512]

                def qk_unit(wc, dst, p, wi):
                    pc = pqc.tile([128, 512], F32, name="pqk")
                    for ck in range(6):
                        nc.tensor.matmul(
                            pc[:],
                            wc[:, ck * 384 + p * 128:ck * 384 + (p + 1) * 128],
                            xcol[:, ck * 512:(ck + 1) * 512],
                            start=(ck == 0), stop=(ck == 5))
                    dsl = dst[p][:, tt * 512:(tt + 1) * 512]
                    uh = ru.tile([128, 512], F16)
                    uhs = ru.tile([128, 512], F16)
                    with nc.allow_low_precision(reason="f16 q/k"):
                        nc.vector.tensor_mul(dsl, pc[:], csl)
                        if (wi + p) % 2 == 0:
                            nc.gpsimd.tensor_mul(uh[:], pc[:], ssl)
                        else:
                            nc.vector.tensor_mul(uh[:], pc[:], ssl)
                        nc.vector.stream_shuffle(uhs[:], uh[:], SHUF_MASK)
                        nc.vector.tensor_add(dsl, dsl, uhs[:])

                def v_unit(tj):
                    tb = tt * 4 + tj
                    pvt = pqc.tile([128, 512], F32, name="pqk")
                    for ck in range(6):
                        nc.tensor.matmul(
                            pvt[:, 0:VW],
                            xcol[:, ck * 512 + tj * 128:
                                    ck * 512 + tj * 128 + 128],
                            sb_wv[:, ck * VW:(ck + 1) * VW],
                            start=(ck == 0), stop=(ck == 5))
                    with nc.allow_low_precision(reason="f16 V store"):
                        nc.gpsimd.tensor_copy(
                            sb_v[:, tb * 6:(tb + 1) * 6, :],
                            pvt[:, 0:VW].rearrange("p (a b) -> p a b", b=65))
                    nc.gpsimd.memset(sb_v[:, tb * 6:(tb + 1) * 6, 64:65], 1.0)

                units = []
                for wi, (wc, dst) in enumerate(((sb_wq, sb_q), (sb_wk, sb_k))):
                    for p in range(PAIRS):
                        units.append(lambda wc=wc, dst=dst, p=p, wi=wi:
                                     qk_unit(wc, dst, p, wi))
                for tj in range(4):
                    units.append(lambda tj=tj: v_unit(tj))
                return units

            def att_units(qt):
                """Attention + O-projection for q block qt as emit-units,
                pipelined so head h's scores precede head h-1's context."""
                ctxq = [cxq.tile([128, 384], F16, name=f"cxq{i}")
                        for i in range(4)]
                etiles = [etp.tile([128, 16 * 512], F16, name="et")
                          for _ in range(HPC)]
                pcts = [None] * HPC

                def sp_unit(h, kb2):
                    # two full k-blocks through one 1024-col PSUM tile
                    p, off = h // 2, (h % 2) * 64
                    et = etiles[h]
                    psc = pps.tile([128, 1024], F32, name="psc")
                    for u in range(2):
                        kb = 2 * kb2 + u
                        nc.tensor.matmul(
                            psc[:, u * 512:(u + 1) * 512],
                            sb_k[p][off:off + 64, kb * 128:(kb + 1) * 128],
                            sb_q[p][off:off + 64, qt * 512:(qt + 1) * 512],
                            start=True, stop=True)
                    with nc.allow_low_precision(reason="f16 attn"):
                        nc.scalar.activation(
                            et[:, kb2 * 1024:(kb2 + 1) * 1024], psc[:],
                            mybir.ActivationFunctionType.Exp)

                def sd_unit(h, jj):
                    # two diagonal k-blocks (2jj, 2jj+1), masked after exp
                    p, off = h // 2, (h % 2) * 64
                    et = etiles[h]
                    psc = pps.tile([128, 1024], F32, name="psc")
                    for u in range(2):
                        j = 2 * jj + u
                        kb = 4 * qt + j
                        lo = j * 128
                        nc.tensor.matmul(
                            psc[:, u * 512 + lo:(u + 1) * 512],
                            sb_k[p][off:off + 64, kb * 128:(kb + 1) * 128],
                            sb_q[p][off:off + 64,
                                    qt * 512 + lo:(qt + 1) * 512],
                            start=True, stop=True)
                    with nc.allow_low_precision(reason="f16 attn"):
                        for u in range(2):
                            j = 2 * jj + u
                            kb = 4 * qt + j
                            lo = j * 128
                            esl = et[:, kb * 512 + lo:(kb + 1) * 512]
                            nc.scalar.activation(
                                esl, psc[:, u * 512 + lo:(u + 1) * 512],
                                mybir.ActivationFunctionType.Exp)
                            msl = et[:, kb * 512 + lo:kb * 512 + lo + 128]
                            nc.vector.tensor_mul(msl, msl, sb_mask[:])

                def ctx_unit(h, qc):
                    # flipped context: [128q, 65] accumulation, 65-col slots
                    # of a shared PSUM bank
                    if qc == 0:
                        pcts[h] = pcx.tile([128, 260], F32, name="pct")
                    pct = pcts[h]
                    et = etiles[h]
                    qg = 4 * qt + qc
                    psl = pct[:, qc * 65:(qc + 1) * 65]
                    for kb in range(qg + 1):
                        nc.tensor.matmul(
                            psl,
                            et[:, kb * 512 + qc * 128:kb * 512 + qc * 128 + 128],
                            sb_v[:, kb * 6 + h, :],
                            start=(kb == 0), stop=(kb == qg),
                            skip_group_check=True)
                    rc = nrm.tile([128, 1], F32)
                    nc.vector.reciprocal(rc[:], pct[:, qc * 65 + 64:
                                                       qc * 65 + 65])
                    with nc.allow_low_precision(reason="f16 ctx"):
                        nc.vector.tensor_scalar_mul(
                            out=ctxq[qc][:, h * 64:(h + 1) * 64],
                            in0=pct[:, qc * 65:qc * 65 + 64],
                            scalar1=rc[:])

                def to_unit(qc):
                    # transpose ctx [q, d] -> [d, q], then O projection
                    tb = qt * 4 + qc
                    ptr = ptp.tile([128, 384], F16)
                    for p3 in range(PAIRS):
                        nc.tensor.transpose(
                            ptr[:, p3 * 128:(p3 + 1) * 128],
                            ctxq[qc][:, p3 * 128:(p3 + 1) * 128],
                            sb_iden[:])
                        with nc.allow_low_precision(reason="f16 ctxT"):
                            nc.gpsimd.tensor_copy(
                                sb_ctxT[p3][:, tb * 128:(tb + 1) * 128],
                                ptr[:, p3 * 128:(p3 + 1) * 128])
                    obuf = otp.tile([128, D], F32)
                    for half in range(2):
                        po = pps.tile([128, 1024], F32, name="psc")
                        for ci in range(3):
                            nc.tensor.matmul(
                                po[:, 0:384],
                                sb_ctxT[ci][:, tb * 128:(tb + 1) * 128],
                                sb_wo[:, ci * D + half * 384:
                                         ci * D + half * 384 + 384],
                                start=(ci == 0), stop=(ci == 2))
                        nc.vector.tensor_copy(
                            obuf[:, half * 384:(half + 1) * 384],
                            po[:, 0:384])
                    nc.sync.dma_start(
                        out_dram[tb * 128:(tb + 1) * 128, :], obuf[:])

                units = []
                for h in range(HPC):
                    su = [lambda h=h, kb2=kb2: sp_unit(h, kb2)
                          for kb2 in range(2 * qt)]
                    su += [lambda h=h, jj=jj: sd_unit(h, jj) for jj in range(2)]
                    if h == 0:
                        units += su
                    else:
                        cu = [lambda h=h, qc=qc: ctx_unit(h - 1, qc)
                              for qc in range(4)]
                        units += interleave(su, cu)
                units += [lambda qc=qc: ctx_unit(HPC - 1, qc)
                          for qc in range(4)]
                units += [lambda qc=qc: to_unit(qc) for qc in range(4)]
                return units

            # --- main emission: weave attention(qt-1) with proj(tt) -------
            load_x(1)
            for u in proj_units(0):
                u()
            for tt in range(1, 4):
                if tt < 3:
                    load_x(tt + 1)
                # att(tt-1) uses only block-(tt-1) data; proj(tt) fills the
                # PE stalls while Act churns through att's exps
                for u in interleave(att_units(tt - 1), proj_units(tt)):
                    u()
            for u in att_units(3):
                u()

            if with_collective:
                nc.gpsimd.collective_compute(
                    "AllReduce", mybir.AluOpType.add,
                    replica_groups=[[0, 1], [2, 3], [4, 5], [6, 7]],
                    ins=[bounce_in.opt()], outs=[bounce_out.opt()])
                nc.sync.dma_start(out[:], bounce_out[:])
    nc.compile()
    return nc


def make_in_maps(x, w_q, w_k, w_v, w_o, token_positions):
    # RoPE row layout: per 64-dim head-half, rows are 2 quadrant-pairs of
    # [16 even dims | 16 odd dims]; stream_shuffle swaps the 16-row halves
    # within each 32-row quadrant.
    r64 = np.arange(64)
    perm64 = 2 * (16 * (r64 // 32) + (r64 % 16)) + ((r64 % 32) >= 16)
    pos = np.asarray(token_positions).astype(np.float32)
    inv = THETA ** (-np.arange(32, dtype=np.float32) / 32.0)
    ang = inv[:, None] * pos[None, :]                       # [32 freqs, S]
    c32 = np.cos(ang).astype(np.float32)
    s32 = np.sin(ang).astype(np.float32)
    r128 = np.arange(128)
    fi = 16 * ((r128 // 32) % 2) + (r128 % 16)              # freq per row
    sgn = np.where((r128 % 32) < 16, 1.0, -1.0).astype(np.float32)
    cosd = c32[fi].astype(np.float16)                       # [128, S]
    sind = (s32[fi] * sgn[:, None]).astype(np.float16)
    kloc = np.arange(128)[:, None]
    qloc = np.arange(128)[None, :]
    maskd = (kloc <= qloc).astype(np.float16)
    idend = np.eye(128, dtype=np.float16)
    xn = np.asarray(x, dtype=np.float32)
    wqn = np.asarray(w_q, dtype=np.float32)
    wkn = np.asarray(w_k, dtype=np.float32)
    wvn = np.asarray(w_v, dtype=np.float32)
    won = np.asarray(w_o, dtype=np.float32)
    in_maps = []
    for c in range(N_CORES):
        b, hg = c // 2, c % 2
        heads = hg * HPC + np.arange(HPC)
        rows_eo = (heads[:, None] * 64 + perm64[None, :]).reshape(-1)
        wv_r = np.zeros((D, VW), np.float32)
        for h in range(HPC):
            g = hg * HPC + h
            wv_r[:, h * 65:h * 65 + 64] = wvn[g * 64:(g + 1) * 64, :].T
        wo_cols = (heads[:, None] * 64 + np.arange(64)[None, :]).reshape(-1)
        wo_r = np.ascontiguousarray(won[:, wo_cols].T).astype(np.float16)
        in_maps.append({
            "xT": np.ascontiguousarray(xn[b].T),
            "wqT": np.ascontiguousarray((wqn[rows_eo] * 0.125).T),
            "wkT": np.ascontiguousarray(wkn[rows_eo].T),
            "wv": wv_r,
            "wo": wo_r,
            "cos": cosd,
            "sin": sind,
            "mask": maskd,
            "iden": idend,
        })
    return in_maps


def kernel(x, w_q, w_k, w_v, w_o, token_positions):
    global _NC
    if _NC is None:
        _NC = build_nc()
    in_maps = make_in_maps(x, w_q, w_k, w_v, w_o, token_positions)
    res = run_bass_kernel_spmd(_NC, in_maps, core_ids=list(range(N_CORES)))
    return np.stack([res.results[2 * b]["out"] for b in range(B)], axis=0)
